# revision 8
# baseline (speedup 1.0000x reference)
"""Trainium2 Bass kernel for the GeometricSNN problem.

Strategy (8 NeuronCores, batch-sharded, B_local=32 per core):
  - cur1 = x @ W1.T is state-independent -> batched over all T as one big
    matmul per core, computed transposed (cur1.T layout [N_h, T*B_local])
    so the sequential scan runs on [128, 256] tiles with full DVE lanes.
  - Precision: fp16 hi/lo split (scaled residuals) of x and W1/W2, three
    matmul passes (hi*hi accumulated separately from the two cross terms,
    combined as H + C/2048 on evacuation).  This reproduces fp32-accurate
    products (verified: zero spike flips vs the fp32 reference, mem2 rel
    err ~5e-4 which equals the fp32-vs-fp64 noise floor).
  - The hi/lo split and the x transpose happen on the host as part of
    sharding prep; total device DMA bytes equal the fp32 input size.
  - Per chunk of 320 columns (10 time steps x 32 batch): mm1 (3 passes,
    PE), evacuation+combine (ACT scaled copy + DVE add), 10 scan steps x
    3 DVE ops, mm2 (single pass: W2 hi/lo stacked into one stationary,
    spikes are exact in fp16), layer-2 scan.
  - Cost-model timeline: ~589us/core, vs ~533us pure matmul-stream floor
    for this precision scheme (PE-bound; DMA ~73us and DVE ~150us are
    fully hidden).
"""

import sys

sys.path.insert(0, "/opt/trn_rl_repo")

import numpy as np

import concourse.bacc as bacc
import concourse.mybir as mybir
from concourse import tile
from concourse.bass_utils import run_bass_kernel_spmd

T, B, NIN, NH, NOUT = 100, 256, 2048, 1024, 10
NCORES = 8
BL = B // NCORES            # 32 batch per core
COLS = T * BL               # 3200 columns (t-major: col = t*BL + b)
NCH = 10                    # chunks
CC = COLS // NCH            # 320 columns per chunk
SPC = CC // BL              # 10 scan steps per chunk
KT1 = NIN // 128            # 16 contraction tiles for mm1
KT2 = NH // 128             # 8 contraction tiles for mm2
MT = NH // 128              # 8 output row tiles of cur1.T
BETA, TH, SC = 0.9, 1.0, 2048.0

f32 = mybir.dt.float32
f16 = mybir.dt.float16
ALU = mybir.AluOpType

_cache = {}


def _build(use_b1, use_b2):
    nc = bacc.Bacc("TRN2", target_bir_lowering=False, debug=False)
    xh_d = nc.dram_tensor("xh", [NIN, COLS], f16, kind="ExternalInput").ap()
    xl_d = nc.dram_tensor("xl", [NIN, COLS], f16, kind="ExternalInput").ap()
    w1h_d = nc.dram_tensor("w1h", [NIN, NH], f16, kind="ExternalInput").ap()
    w1l_d = nc.dram_tensor("w1l", [NIN, NH], f16, kind="ExternalInput").ap()
    w2h_d = nc.dram_tensor("w2h", [NH, NOUT], f16, kind="ExternalInput").ap()
    w2l_d = nc.dram_tensor("w2l", [NH, NOUT], f16, kind="ExternalInput").ap()
    if use_b1:
        b1_d = nc.dram_tensor("b1t", [128, MT], f32, kind="ExternalInput").ap()
    if use_b2:
        b2_d = nc.dram_tensor("b2t", [NOUT, 1], f32, kind="ExternalInput").ap()
    spk_d = nc.dram_tensor("spk2t", [NOUT, COLS], f32, kind="ExternalOutput").ap()
    mem2_d = nc.dram_tensor("mem2t", [NOUT, BL], f32, kind="ExternalOutput").ap()

    with tile.TileContext(nc) as tc:
        with tc.tile_pool(name="wp", bufs=1) as wp, \
             tc.tile_pool(name="xp", bufs=2) as xp, \
             tc.tile_pool(name="cp", bufs=2) as cp, \
             tc.tile_pool(name="sp", bufs=2) as sp, \
             tc.tile_pool(name="st", bufs=1) as st, \
             tc.tile_pool(name="scr", bufs=2) as scr, \
             tc.tile_pool(name="pp1", bufs=2, space="PSUM") as pp1, \
             tc.tile_pool(name="pp2", bufs=1, space="PSUM") as pp2:

            w1h = wp.tile([128, KT1 * NH], f16, tag="w1h")
            w1l = wp.tile([128, KT1 * NH], f16, tag="w1l")
            for k in range(KT1):
                nc.sync.dma_start(w1h[:, k * NH:(k + 1) * NH],
                                  w1h_d[k * 128:(k + 1) * 128, :])
                nc.sync.dma_start(w1l[:, k * NH:(k + 1) * NH],
                                  w1l_d[k * 128:(k + 1) * 128, :])
            # W2 hi and lo stacked into one stationary per K-tile so mm2 runs
            # as a single matmul stream.  The lo block sits at column 32 (not
            # 10) because engines may only read PSUM at 32-aligned partition
            # offsets; filler columns are zeroed.
            w2s = wp.tile([128, KT2 * 64], f16, tag="w2s")
            nc.vector.memset(w2s[:], 0.0)
            for k in range(KT2):
                nc.sync.dma_start(w2s[:, k * 64:k * 64 + NOUT],
                                  w2h_d[k * 128:(k + 1) * 128, :])
                nc.sync.dma_start(w2s[:, k * 64 + 32:k * 64 + 32 + NOUT],
                                  w2l_d[k * 128:(k + 1) * 128, :])
            if use_b1:
                b1sb = wp.tile([128, MT], f32, tag="b1")
                nc.sync.dma_start(b1sb[:], b1_d)
            if use_b2:
                b2sb = wp.tile([NOUT, 1], f32, tag="b2")
                nc.sync.dma_start(b2sb[:], b2_d)

            mem1 = st.tile([128, MT * BL], f32, tag="mem1")
            nc.vector.memset(mem1[:], 0.0)
            mem2 = st.tile([NOUT, BL], f32, tag="mem2")
            nc.vector.memset(mem2[:], 0.0)
            zspk = st.tile([128, MT * BL], f16, tag="zspk")
            nc.vector.memset(zspk[:], 0.0)
            cur2 = st.tile([NOUT, COLS], f32, tag="cur2")
            outs = st.tile([NOUT, COLS + BL], f32, tag="outs")
            nc.vector.memset(outs[:, :BL], 0.0)

            prev_spk = zspk[:]
            for c in range(NCH):
                xh_t = xp.tile([128, KT1 * CC], f16, tag="xh")
                xl_t = xp.tile([128, KT1 * CC], f16, tag="xl")
                for k in range(KT1):
                    nc.sync.dma_start(xh_t[:, k * CC:(k + 1) * CC],
                                      xh_d[k * 128:(k + 1) * 128, c * CC:(c + 1) * CC])
                    nc.sync.dma_start(xl_t[:, k * CC:(k + 1) * CC],
                                      xl_d[k * 128:(k + 1) * 128, c * CC:(c + 1) * CC])
                cur1 = cp.tile([128, MT * CC], f32, tag="cur1")
                for m in range(MT):
                    ph = pp1.tile([128, CC], f32, tag="ph")
                    pc = pp1.tile([128, CC], f32, tag="pc")
                    ws = slice(m * 128, (m + 1) * 128)
                    for k in range(KT1):
                        nc.tensor.matmul(
                            ph[:], w1h[:, k * NH + m * 128:k * NH + (m + 1) * 128],
                            xh_t[:, k * CC:(k + 1) * CC],
                            start=(k == 0), stop=(k == KT1 - 1))
                    for k in range(KT1):
                        nc.tensor.matmul(
                            pc[:], w1l[:, k * NH + m * 128:k * NH + (m + 1) * 128],
                            xh_t[:, k * CC:(k + 1) * CC],
                            start=(k == 0), stop=False)
                        nc.tensor.matmul(
                            pc[:], w1h[:, k * NH + m * 128:k * NH + (m + 1) * 128],
                            xl_t[:, k * CC:(k + 1) * CC],
                            start=False, stop=(k == KT1 - 1))
                    dst = cur1[:, m * CC:(m + 1) * CC]
                    tmp = scr.tile([128, CC], f32, tag="evac")
                    nc.scalar.mul(tmp[:], pc[:], 1.0 / SC)
                    nc.vector.tensor_tensor(dst, tmp[:], ph[:], op=ALU.add)
                    if use_b1:
                        nc.vector.tensor_scalar_add(dst, dst, b1sb[:, m:m + 1])

                spk_t = sp.tile([128, SPC * MT * BL], f16, tag="spk")
                cur3 = cur1[:].rearrange("p (m w) -> p m w", w=CC)
                for j in range(SPC):
                    u = scr.tile([128, MT * BL], f32, tag="u")
                    nc.vector.scalar_tensor_tensor(
                        u[:].rearrange("p (m b) -> p m b", b=BL),
                        mem1[:].rearrange("p (m b) -> p m b", b=BL), BETA,
                        cur3[:, :, j * BL:(j + 1) * BL],
                        op0=ALU.mult, op1=ALU.add)
                    nc.vector.tensor_tensor(mem1[:], u[:], prev_spk,
                                            op=ALU.subtract)
                    spk_sl = spk_t[:, j * MT * BL:(j + 1) * MT * BL]
                    nc.vector.tensor_scalar(spk_sl, mem1[:], TH, None,
                                            op0=ALU.is_gt)
                    prev_spk = spk_sl

                p2 = pp2.tile([64, CC], f32, tag="p2")
                spk3 = spk_t[:].rearrange("p (j q) -> p j q", q=MT * BL)
                for k in range(KT2):
                    rhs = spk3[:, :, k * BL:(k + 1) * BL]
                    nc.tensor.matmul(p2[:], w2s[:, k * 64:(k + 1) * 64],
                                     rhs, start=(k == 0), stop=(k == KT2 - 1))
                dst2 = cur2[:, c * CC:(c + 1) * CC]
                tmp2 = scr.tile([NOUT, CC], f32, tag="evac2")
                nc.scalar.mul(tmp2[:], p2[32:32 + NOUT, :], 1.0 / SC)
                nc.vector.tensor_tensor(dst2, tmp2[:], p2[0:NOUT, :], op=ALU.add)
                if use_b2:
                    nc.vector.tensor_scalar_add(dst2, dst2, b2sb[:, 0:1])

                for j in range(SPC):
                    t = c * SPC + j
                    u2 = scr.tile([NOUT, BL], f32, tag="u2")
                    nc.vector.scalar_tensor_tensor(
                        u2[:], mem2[:], BETA, cur2[:, t * BL:(t + 1) * BL],
                        op0=ALU.mult, op1=ALU.add)
                    nc.vector.tensor_tensor(mem2[:], u2[:],
                                            outs[:, t * BL:(t + 1) * BL],
                                            op=ALU.subtract)
                    nc.vector.tensor_scalar(outs[:, (t + 1) * BL:(t + 2) * BL],
                                            mem2[:], TH, None, op0=ALU.is_gt)

            nc.sync.dma_start(spk_d, outs[:, BL:])
            nc.sync.dma_start(mem2_d, mem2[:])

    nc.compile()
    return nc


def _get(use_b1, use_b2):
    key = (use_b1, use_b2)
    if key not in _cache:
        _cache[key] = _build(use_b1, use_b2)
    return _cache[key]


def _split16(a):
    hi = a.astype(np.float16)
    lo = ((a - hi.astype(np.float32)) * np.float32(SC)).astype(np.float16)
    return hi, lo


def kernel(x, W1, b1, W2, b2):
    x = np.asarray(x, np.float32)
    W1 = np.asarray(W1, np.float32)
    W2 = np.asarray(W2, np.float32)
    b1 = np.asarray(b1, np.float32)
    b2 = np.asarray(b2, np.float32)

    xh, xl = _split16(x)
    w1h, w1l = _split16(np.ascontiguousarray(W1.T))
    w2h, w2l = _split16(np.ascontiguousarray(W2.T))
    use_b1 = bool(np.any(b1))
    use_b2 = bool(np.any(b2))
    nc = _get(use_b1, use_b2)

    in_maps = []
    for c in range(NCORES):
        sl = slice(c * BL, (c + 1) * BL)
        m = {
            "xh": np.ascontiguousarray(xh[:, sl, :].transpose(2, 0, 1)).reshape(NIN, COLS),
            "xl": np.ascontiguousarray(xl[:, sl, :].transpose(2, 0, 1)).reshape(NIN, COLS),
            "w1h": w1h, "w1l": w1l, "w2h": w2h, "w2l": w2l,
        }
        if use_b1:
            m["b1t"] = np.ascontiguousarray(b1.reshape(MT, 128).T)
        if use_b2:
            m["b2t"] = np.ascontiguousarray(b2.reshape(NOUT, 1))
        in_maps.append(m)

    res = run_bass_kernel_spmd(nc, in_maps, list(range(NCORES)))

    spk = np.empty((T, B, NOUT), np.float32)
    mem2 = np.empty((B, NOUT), np.float32)
    for c in range(NCORES):
        o = res.results[c]
        spk[:, c * BL:(c + 1) * BL, :] = (
            o["spk2t"].reshape(NOUT, T, BL).transpose(1, 2, 0))
        mem2[c * BL:(c + 1) * BL, :] = o["mem2t"].T
    return spk, mem2


# revision 11
# speedup vs baseline: 1.0035x; 1.0035x over previous
"""Trainium2 Bass kernel for the GeometricSNN problem.

Strategy (8 NeuronCores, batch-sharded, B_local=32 per core):
  - cur1 = x @ W1.T is state-independent -> batched over all T as one big
    matmul per core, computed transposed (cur1.T layout [N_h, T*B_local])
    so the sequential scan runs on [128, 256] tiles with full DVE lanes.
  - Precision: fp16 hi/lo split (scaled residuals) of x and W1/W2, three
    matmul passes (hi*hi accumulated separately from the two cross terms,
    combined as H + C/2048 on evacuation).  This reproduces fp32-accurate
    products (verified: zero spike flips vs the fp32 reference, mem2 rel
    err ~5e-4 which equals the fp32-vs-fp64 noise floor).
  - The hi/lo split and the x transpose happen on the host as part of
    sharding prep; total device DMA bytes equal the fp32 input size.
  - Per chunk of 320 columns (10 time steps x 32 batch): mm1 (3 passes,
    PE), evacuation+combine (ACT scaled copy + DVE add), 10 scan steps x
    3 DVE ops, mm2 (single pass: W2 hi/lo stacked into one stationary,
    spikes are exact in fp16), layer-2 scan.
  - Cost-model timeline: ~589us/core, vs ~533us pure matmul-stream floor
    for this precision scheme (PE-bound; DMA ~73us and DVE ~150us are
    fully hidden).
"""

import sys

sys.path.insert(0, "/opt/trn_rl_repo")

import numpy as np

import concourse.bacc as bacc
import concourse.mybir as mybir
from concourse import tile
from concourse.bass_utils import run_bass_kernel_spmd

T, B, NIN, NH, NOUT = 100, 256, 2048, 1024, 10
NCORES = 8
BL = B // NCORES            # 32 batch per core
COLS = T * BL               # 3200 columns (t-major: col = t*BL + b)
NCH = 10                    # chunks
CC = COLS // NCH            # 320 columns per chunk
SPC = CC // BL              # 10 scan steps per chunk
KT1 = NIN // 128            # 16 contraction tiles for mm1
KT2 = NH // 128             # 8 contraction tiles for mm2
MT = NH // 128              # 8 output row tiles of cur1.T
BETA, TH, SC = 0.9, 1.0, 2048.0

f32 = mybir.dt.float32
f16 = mybir.dt.float16
ALU = mybir.AluOpType

_cache = {}


def _build(use_b1, use_b2):
    nc = bacc.Bacc("TRN2", target_bir_lowering=False, debug=False)
    xh_d = nc.dram_tensor("xh", [NIN, COLS], f16, kind="ExternalInput").ap()
    xl_d = nc.dram_tensor("xl", [NIN, COLS], f16, kind="ExternalInput").ap()
    w1h_d = nc.dram_tensor("w1h", [NIN, NH], f16, kind="ExternalInput").ap()
    w1l_d = nc.dram_tensor("w1l", [NIN, NH], f16, kind="ExternalInput").ap()
    w2h_d = nc.dram_tensor("w2h", [NH, NOUT], f16, kind="ExternalInput").ap()
    w2l_d = nc.dram_tensor("w2l", [NH, NOUT], f16, kind="ExternalInput").ap()
    if use_b1:
        b1_d = nc.dram_tensor("b1t", [128, MT], f32, kind="ExternalInput").ap()
    if use_b2:
        b2_d = nc.dram_tensor("b2t", [NOUT, 1], f32, kind="ExternalInput").ap()
    spk_d = nc.dram_tensor("spk2t", [NOUT, COLS], f32, kind="ExternalOutput").ap()
    mem2_d = nc.dram_tensor("mem2t", [NOUT, BL], f32, kind="ExternalOutput").ap()

    with tile.TileContext(nc) as tc:
        with tc.tile_pool(name="wp", bufs=1) as wp, \
             tc.tile_pool(name="xp", bufs=2) as xp, \
             tc.tile_pool(name="cp", bufs=2) as cp, \
             tc.tile_pool(name="sp", bufs=2) as sp, \
             tc.tile_pool(name="st", bufs=1) as st, \
             tc.tile_pool(name="scr", bufs=2) as scr, \
             tc.tile_pool(name="pp1", bufs=2, space="PSUM") as pp1, \
             tc.tile_pool(name="pp2", bufs=1, space="PSUM") as pp2:

            w1h = wp.tile([128, KT1 * NH], f16, tag="w1h")
            w1l = wp.tile([128, KT1 * NH], f16, tag="w1l")
            # hi tiles first: chunk 0 runs all its hi passes before the
            # cross passes, so w1l only needs to arrive ~12us later
            for k in range(KT1):
                nc.sync.dma_start(w1h[:, k * NH:(k + 1) * NH],
                                  w1h_d[k * 128:(k + 1) * 128, :])
            for k in range(KT1):
                nc.sync.dma_start(w1l[:, k * NH:(k + 1) * NH],
                                  w1l_d[k * 128:(k + 1) * 128, :])
            # W2 hi and lo stacked into one stationary per K-tile so mm2 runs
            # as a single matmul stream.  The lo block sits at column 32 (not
            # 10) because engines may only read PSUM at 32-aligned partition
            # offsets; filler columns are zeroed.
            w2s = wp.tile([128, KT2 * 64], f16, tag="w2s")
            nc.vector.memset(w2s[:], 0.0)
            for k in range(KT2):
                nc.sync.dma_start(w2s[:, k * 64:k * 64 + NOUT],
                                  w2h_d[k * 128:(k + 1) * 128, :])
                nc.sync.dma_start(w2s[:, k * 64 + 32:k * 64 + 32 + NOUT],
                                  w2l_d[k * 128:(k + 1) * 128, :])
            if use_b1:
                b1sb = wp.tile([128, MT], f32, tag="b1")
                nc.sync.dma_start(b1sb[:], b1_d)
            if use_b2:
                b2sb = wp.tile([NOUT, 1], f32, tag="b2")
                nc.sync.dma_start(b2sb[:], b2_d)

            mem1 = st.tile([128, MT * BL], f32, tag="mem1")
            nc.vector.memset(mem1[:], 0.0)
            mem2 = st.tile([NOUT, BL], f32, tag="mem2")
            nc.vector.memset(mem2[:], 0.0)
            zspk = st.tile([128, MT * BL], f16, tag="zspk")
            nc.vector.memset(zspk[:], 0.0)
            cur2 = st.tile([NOUT, COLS], f32, tag="cur2")
            outs = st.tile([NOUT, COLS + BL], f32, tag="outs")
            nc.vector.memset(outs[:, :BL], 0.0)

            prev_spk = zspk[:]
            for c in range(NCH):
                xh_t = xp.tile([128, KT1 * CC], f16, tag="xh")
                xl_t = xp.tile([128, KT1 * CC], f16, tag="xl")
                for k in range(KT1):
                    nc.sync.dma_start(xh_t[:, k * CC:(k + 1) * CC],
                                      xh_d[k * 128:(k + 1) * 128, c * CC:(c + 1) * CC])
                for k in range(KT1):
                    nc.sync.dma_start(xl_t[:, k * CC:(k + 1) * CC],
                                      xl_d[k * 128:(k + 1) * 128, c * CC:(c + 1) * CC])
                cur1 = cp.tile([128, MT * CC], f32, tag="cur1")
                if c == 0:
                    # startup: run every hi pass first (only needs w1h + xh),
                    # parking results in SBUF, so the cross passes start
                    # exactly when w1l lands instead of stalling per-m
                    hbuf = cp.tile([128, MT * CC], f32, tag="hbuf")
                    for m in range(MT):
                        ph = pp1.tile([128, CC], f32, tag="ph")
                        for k in range(KT1):
                            nc.tensor.matmul(
                                ph[:], w1h[:, k * NH + m * 128:k * NH + (m + 1) * 128],
                                xh_t[:, k * CC:(k + 1) * CC],
                                start=(k == 0), stop=(k == KT1 - 1))
                        nc.scalar.copy(hbuf[:, m * CC:(m + 1) * CC], ph[:])
                    for m in range(MT):
                        pc = pp1.tile([128, CC], f32, tag="pc")
                        for k in range(KT1):
                            nc.tensor.matmul(
                                pc[:], w1l[:, k * NH + m * 128:k * NH + (m + 1) * 128],
                                xh_t[:, k * CC:(k + 1) * CC],
                                start=(k == 0), stop=False)
                            nc.tensor.matmul(
                                pc[:], w1h[:, k * NH + m * 128:k * NH + (m + 1) * 128],
                                xl_t[:, k * CC:(k + 1) * CC],
                                start=False, stop=(k == KT1 - 1))
                        dst = cur1[:, m * CC:(m + 1) * CC]
                        tmp = scr.tile([128, CC], f32, tag="evac")
                        nc.scalar.mul(tmp[:], pc[:], 1.0 / SC)
                        nc.vector.tensor_tensor(dst, tmp[:],
                                                hbuf[:, m * CC:(m + 1) * CC],
                                                op=ALU.add)
                        if use_b1:
                            nc.vector.tensor_scalar_add(dst, dst, b1sb[:, m:m + 1])
                else:
                 for m in range(MT):
                    ph = pp1.tile([128, CC], f32, tag="ph")
                    pc = pp1.tile([128, CC], f32, tag="pc")
                    ws = slice(m * 128, (m + 1) * 128)
                    for k in range(KT1):
                        nc.tensor.matmul(
                            ph[:], w1h[:, k * NH + m * 128:k * NH + (m + 1) * 128],
                            xh_t[:, k * CC:(k + 1) * CC],
                            start=(k == 0), stop=(k == KT1 - 1))
                    for k in range(KT1):
                        nc.tensor.matmul(
                            pc[:], w1l[:, k * NH + m * 128:k * NH + (m + 1) * 128],
                            xh_t[:, k * CC:(k + 1) * CC],
                            start=(k == 0), stop=False)
                        nc.tensor.matmul(
                            pc[:], w1h[:, k * NH + m * 128:k * NH + (m + 1) * 128],
                            xl_t[:, k * CC:(k + 1) * CC],
                            start=False, stop=(k == KT1 - 1))
                    dst = cur1[:, m * CC:(m + 1) * CC]
                    tmp = scr.tile([128, CC], f32, tag="evac")
                    nc.scalar.mul(tmp[:], pc[:], 1.0 / SC)
                    nc.vector.tensor_tensor(dst, tmp[:], ph[:], op=ALU.add)
                    if use_b1:
                        nc.vector.tensor_scalar_add(dst, dst, b1sb[:, m:m + 1])

                spk_t = sp.tile([128, SPC * MT * BL], f16, tag="spk")
                cur3 = cur1[:].rearrange("p (m w) -> p m w", w=CC)
                for j in range(SPC):
                    u = scr.tile([128, MT * BL], f32, tag="u")
                    nc.vector.scalar_tensor_tensor(
                        u[:].rearrange("p (m b) -> p m b", b=BL),
                        mem1[:].rearrange("p (m b) -> p m b", b=BL), BETA,
                        cur3[:, :, j * BL:(j + 1) * BL],
                        op0=ALU.mult, op1=ALU.add)
                    nc.vector.tensor_tensor(mem1[:], u[:], prev_spk,
                                            op=ALU.subtract)
                    spk_sl = spk_t[:, j * MT * BL:(j + 1) * MT * BL]
                    nc.vector.tensor_scalar(spk_sl, mem1[:], TH, None,
                                            op0=ALU.is_gt)
                    prev_spk = spk_sl

                p2 = pp2.tile([64, CC], f32, tag="p2")
                spk3 = spk_t[:].rearrange("p (j q) -> p j q", q=MT * BL)
                for k in range(KT2):
                    rhs = spk3[:, :, k * BL:(k + 1) * BL]
                    nc.tensor.matmul(p2[:], w2s[:, k * 64:(k + 1) * 64],
                                     rhs, start=(k == 0), stop=(k == KT2 - 1))
                dst2 = cur2[:, c * CC:(c + 1) * CC]
                tmp2 = scr.tile([NOUT, CC], f32, tag="evac2")
                nc.scalar.mul(tmp2[:], p2[32:32 + NOUT, :], 1.0 / SC)
                nc.vector.tensor_tensor(dst2, tmp2[:], p2[0:NOUT, :], op=ALU.add)
                if use_b2:
                    nc.vector.tensor_scalar_add(dst2, dst2, b2sb[:, 0:1])

                for j in range(SPC):
                    t = c * SPC + j
                    u2 = scr.tile([NOUT, BL], f32, tag="u2")
                    nc.vector.scalar_tensor_tensor(
                        u2[:], mem2[:], BETA, cur2[:, t * BL:(t + 1) * BL],
                        op0=ALU.mult, op1=ALU.add)
                    nc.vector.tensor_tensor(mem2[:], u2[:],
                                            outs[:, t * BL:(t + 1) * BL],
                                            op=ALU.subtract)
                    nc.vector.tensor_scalar(outs[:, (t + 1) * BL:(t + 2) * BL],
                                            mem2[:], TH, None, op0=ALU.is_gt)

            nc.sync.dma_start(spk_d, outs[:, BL:])
            nc.sync.dma_start(mem2_d, mem2[:])

    nc.compile()
    return nc


def _get(use_b1, use_b2):
    key = (use_b1, use_b2)
    if key not in _cache:
        _cache[key] = _build(use_b1, use_b2)
    return _cache[key]


def _split16(a):
    hi = a.astype(np.float16)
    lo = ((a - hi.astype(np.float32)) * np.float32(SC)).astype(np.float16)
    return hi, lo


def kernel(x, W1, b1, W2, b2):
    x = np.asarray(x, np.float32)
    W1 = np.asarray(W1, np.float32)
    W2 = np.asarray(W2, np.float32)
    b1 = np.asarray(b1, np.float32)
    b2 = np.asarray(b2, np.float32)

    xh, xl = _split16(x)
    w1h, w1l = _split16(np.ascontiguousarray(W1.T))
    w2h, w2l = _split16(np.ascontiguousarray(W2.T))
    use_b1 = bool(np.any(b1))
    use_b2 = bool(np.any(b2))
    nc = _get(use_b1, use_b2)

    in_maps = []
    for c in range(NCORES):
        sl = slice(c * BL, (c + 1) * BL)
        m = {
            "xh": np.ascontiguousarray(xh[:, sl, :].transpose(2, 0, 1)).reshape(NIN, COLS),
            "xl": np.ascontiguousarray(xl[:, sl, :].transpose(2, 0, 1)).reshape(NIN, COLS),
            "w1h": w1h, "w1l": w1l, "w2h": w2h, "w2l": w2l,
        }
        if use_b1:
            m["b1t"] = np.ascontiguousarray(b1.reshape(MT, 128).T)
        if use_b2:
            m["b2t"] = np.ascontiguousarray(b2.reshape(NOUT, 1))
        in_maps.append(m)

    res = run_bass_kernel_spmd(nc, in_maps, list(range(NCORES)))

    spk = np.empty((T, B, NOUT), np.float32)
    mem2 = np.empty((B, NOUT), np.float32)
    for c in range(NCORES):
        o = res.results[c]
        spk[:, c * BL:(c + 1) * BL, :] = (
            o["spk2t"].reshape(NOUT, T, BL).transpose(1, 2, 0))
        mem2[c * BL:(c + 1) * BL, :] = o["mem2t"].T
    return spk, mem2
